# revision 9
# baseline (speedup 1.0000x reference)
"""Trainium2 Bass kernel for BatchEmbeddingUpdater (gnn_message_passing).

Semantics replicated (matching the jax reference with in-order scatters):
    src_emb = (prev[src] + src_nbr @ W_nig.T + b_nig) @ W_node.T + b_node + prev[src]
    dst_emb = (prev[dst] + dst_nbr @ W_nig.T + b_nig) @ W_node.T + b_node + prev[dst]
    out = prev;  out[src] = src_emb;  out[dst] = dst_emb
(duplicates: LAST write wins within a batch; dst beats src — XLA/numpy
in-order scatter semantics)

Algebraic fusion (host precompute):
    out_row = prev_row + delta_row
    delta_row = prev_row @ Wn + nbr_row @ Wc + bc
    with Wn = W_node.T, Wc = W_nig.T @ W_node.T, bc = b_nig @ W_node.T + b_node

Sharding: previous_embedding row-partitioned across 8 cores (125k rows).
Winner updates are routed on host to the owning core (gather row == scatter
row). Each core's shard is split into 8 zones (15625 rows, separate output
DRAM tensors): zone-local row indices fit int16 for dma_gather /
dma_scatter_add, and the bulk copy -> scatter ordering is per-zone.

Device per zone: bulk DRAM->DRAM copy of the zone; dma_gather of the zone's
update rows; per 512-update group: PE transpose -> bf16 hi/lo split ->
6 wide bf16 matmuls (Wn_h@Gh + Wn_h@Gl + Wn_l@Gh + Wc_h@Nh + Wc_h@Nl +
Wc_l@Nh) -> +bc -> PE transpose back -> x mask (zeroes pad slots); then
dma_scatter_add of the deltas onto the copied rows. bf16 hi+lo carries
~17 significand bits -> ~1e-5 relative error; the residual "+prev" is the
exact DRAM row via the scatter-add.
"""

import numpy as np

N_NODES = 1_000_000
BATCH = 100_000
D = 128
N_CORES = 8
RPC = N_NODES // N_CORES        # 125_000 rows per core
N_ZONES = 8
RPZ = RPC // N_ZONES            # 15_625 rows per zone (int16-addressable)
TILES_PER_ZONE = 24
ZONE_CAP = TILES_PER_ZONE * 128  # 3072 padded updates per zone
T_TILES = N_ZONES * TILES_PER_ZONE  # 192
CAP = N_ZONES * ZONE_CAP        # 24_576 updates per core (padded)
GRP = 4                          # tiles per matmul group (512 updates)
GROUPS_PER_ZONE = TILES_PER_ZONE // GRP
IDX_COLS = ZONE_CAP // 16        # 192 int16 idx columns per zone
COPY_CHUNKS = 2                 # bulk-copy DMAs per zone

_program = None
last_results = None  # BassKernelResults of the most recent kernel() call


def build_program():
    """Build + compile the (single, SPMD) Bass program. Cached."""
    global _program
    if _program is not None:
        return _program

    import concourse.mybir as mybir
    import concourse.tile as tile
    from concourse import bacc
    from concourse.masks import make_identity
    from concourse.tile_rust import add_dep_helper

    f32 = mybir.dt.float32
    bf16 = mybir.dt.bfloat16
    i16 = mybir.dt.int16
    AluOp = mybir.AluOpType

    nc = bacc.Bacc("TRN2", target_bir_lowering=False, debug=False,
                   num_devices=N_CORES)

    prev = nc.dram_tensor("prev", [RPC, D], f32, kind="ExternalInput").ap()
    nbh_d = nc.dram_tensor("nbh", [D, CAP], bf16, kind="ExternalInput").ap()
    nbl_d = nc.dram_tensor("nbl", [D, CAP], bf16, kind="ExternalInput").ap()
    idx_d = nc.dram_tensor("idx", [128, N_ZONES * IDX_COLS], i16,
                           kind="ExternalInput").ap()
    mask_d = nc.dram_tensor("mask", [128, T_TILES], f32,
                            kind="ExternalInput").ap()
    wn_d = [nc.dram_tensor(f"wn{s}", [D, D], bf16, kind="ExternalInput").ap()
            for s in "hl"]
    wc_d = [nc.dram_tensor(f"wc{s}", [D, D], bf16, kind="ExternalInput").ap()
            for s in "hl"]
    bc_d = nc.dram_tensor("bc", [D, 1], f32, kind="ExternalInput").ap()
    outs = [nc.dram_tensor(f"out{z}", [RPZ, D], f32, kind="ExternalOutput").ap()
            for z in range(N_ZONES)]

    with tile.TileContext(nc) as tc, \
         tc.tile_pool(name="const", bufs=1) as cpool, \
         tc.tile_pool(name="gather", bufs=2) as gpool, \
         tc.tile_pool(name="gt", bufs=2) as gtpool, \
         tc.tile_pool(name="outb", bufs=2) as opool, \
         tc.tile_pool(name="ps_t", bufs=3, space="PSUM") as pst, \
         tc.tile_pool(name="ps_b", bufs=3, space="PSUM") as psb, \
         tc.tile_pool(name="ps_a", bufs=2, space="PSUM") as psa:

        ident = cpool.tile([128, 128], f32, name="ident")
        make_identity(nc, ident[:])
        wn_sb = [cpool.tile([128, 128], bf16, name=f"wn{s}_sb") for s in "hl"]
        wc_sb = [cpool.tile([128, 128], bf16, name=f"wc{s}_sb") for s in "hl"]
        for d_ap, t in zip(wn_d + wc_d, wn_sb + wc_sb):
            nc.sync.dma_start(out=t[:], in_=d_ap)
        bc_sb = cpool.tile([128, 1], f32, name="bc_sb")
        nc.sync.dma_start(out=bc_sb[:], in_=bc_d)
        idx_sb = cpool.tile([128, N_ZONES * IDX_COLS], i16, name="idx_sb")
        nc.sync.dma_start(out=idx_sb[:], in_=idx_d)
        mask_sb = cpool.tile([128, T_TILES], f32, name="mask_sb")
        nc.sync.dma_start(out=mask_sb[:], in_=mask_d)
        nbh_sb = cpool.tile([128, CAP], bf16, name="nbh_sb")
        nbl_sb = cpool.tile([128, CAP], bf16, name="nbl_sb")
        for c in range(2):
            w = CAP // 2
            nc.scalar.dma_start(out=nbh_sb[:, c * w:(c + 1) * w],
                                in_=nbh_d[:, c * w:(c + 1) * w])
            nc.scalar.dma_start(out=nbl_sb[:, c * w:(c + 1) * w],
                                in_=nbl_d[:, c * w:(c + 1) * w])

        # Bulk copy of the shard into the zone outputs (DRAM -> DRAM).
        # Tile does not track DRAM hazards; explicit scatter-after-copy deps
        # are added below.
        copy_insts = [[] for _ in range(N_ZONES)]
        rows_per_chunk = RPZ // COPY_CHUNKS
        for z in range(N_ZONES):
            for c in range(COPY_CHUNKS):
                r0 = c * rows_per_chunk
                r1 = RPZ if c == COPY_CHUNKS - 1 else r0 + rows_per_chunk
                ci = nc.sync.dma_start(out=outs[z][r0:r1, :],
                                       in_=prev[z * RPZ + r0:z * RPZ + r1, :])
                copy_insts[z].append(ci)

        for z in range(N_ZONES):
            # Gather all of this zone's update rows (zone-local int16 idx).
            g = gpool.tile([128, ZONE_CAP], f32, name="g", tag="g")
            nc.gpsimd.dma_gather(
                out_ap=g[:].rearrange("p (c e) -> p c e", e=128),
                in_ap=prev[z * RPZ:(z + 1) * RPZ, :],
                idxs_ap=idx_sb[:, z * IDX_COLS:(z + 1) * IDX_COLS],
                num_idxs=ZONE_CAP, num_idxs_reg=ZONE_CAP, elem_size=128,
                single_packet=False,
            )
            ob = opool.tile([128, ZONE_CAP], f32, name="ob", tag="ob")
            for grp in range(GROUPS_PER_ZONE):
                t0 = z * TILES_PER_ZONE + grp * GRP  # global tile index
                u0 = t0 * 128                        # update offset in core
                gth = gtpool.tile([128, GRP * 128], bf16, name="gth", tag="gth")
                gtl = gtpool.tile([128, GRP * 128], bf16, name="gtl", tag="gtl")
                for j in range(GRP):
                    c0 = (grp * GRP + j) * 128
                    js = slice(j * 128, (j + 1) * 128)
                    tp = pst.tile([128, 128], f32, name="tp", tag="tp")
                    nc.tensor.transpose(tp[:], g[:, c0:c0 + 128], ident[:])
                    nc.vector.tensor_copy(gth[:, js], tp[:])
                    nc.vector.tensor_tensor(gtl[:, js], tp[:], gth[:, js],
                                            op=AluOp.subtract)
                acc = psa.tile([128, GRP * 128], f32, name="acc", tag="acc")
                nbs = slice(u0, u0 + GRP * 128)
                nc.tensor.matmul(acc[:], lhsT=wn_sb[0][:], rhs=gth[:],
                                 start=True, stop=False)
                nc.tensor.matmul(acc[:], lhsT=wn_sb[0][:], rhs=gtl[:],
                                 start=False, stop=False)
                nc.tensor.matmul(acc[:], lhsT=wn_sb[1][:], rhs=gth[:],
                                 start=False, stop=False)
                nc.tensor.matmul(acc[:], lhsT=wc_sb[0][:], rhs=nbh_sb[:, nbs],
                                 start=False, stop=False)
                nc.tensor.matmul(acc[:], lhsT=wc_sb[0][:], rhs=nbl_sb[:, nbs],
                                 start=False, stop=False)
                nc.tensor.matmul(acc[:], lhsT=wc_sb[1][:], rhs=nbh_sb[:, nbs],
                                 start=False, stop=True)
                outt = gtpool.tile([128, GRP * 128], f32, name="outt",
                                   tag="outt")
                nc.vector.tensor_scalar_add(outt[:], acc[:], bc_sb[:, :1])
                for j in range(GRP):
                    c0 = (grp * GRP + j) * 128
                    tb = psb.tile([128, 128], f32, name="tb", tag="tb")
                    nc.tensor.transpose(tb[:], outt[:, j * 128:(j + 1) * 128],
                                        ident[:])
                    # mask: 1.0 for real updates, 0.0 for pad slots
                    nc.vector.tensor_scalar_mul(
                        ob[:, c0:c0 + 128], tb[:],
                        mask_sb[:, t0 + j:t0 + j + 1])
            # Scatter-add the zone's deltas onto the copied rows.
            sc = nc.gpsimd.dma_scatter_add(
                out_ap=outs[z],
                in_ap=ob[:].rearrange("p (c e) -> p c e", e=128),
                idxs_ap=idx_sb[:, z * IDX_COLS:(z + 1) * IDX_COLS],
                num_idxs=ZONE_CAP, num_idxs_reg=ZONE_CAP, elem_size=128,
                single_packet=False,
            )
            for ci in copy_insts[z]:
                add_dep_helper(sc.ins, ci.ins, sync=True,
                               reason="scatter-add after zone bulk copy")

    nc.compile()
    _program = nc
    return nc


def route_updates(src_ids, dst_ids, src_nbr, dst_nbr):
    """Dedup the two scatter batches into winner updates (last wins, dst
    over src) and return (uniq_node_ids_sorted, winner_nbr_rows)."""
    ids = np.concatenate([np.asarray(src_ids, np.int64),
                          np.asarray(dst_ids, np.int64)])
    rev = ids[::-1]
    uniq, idx_rev = np.unique(rev, return_index=True)
    win = ids.size - 1 - idx_rev        # winning write position
    nbr = np.empty((uniq.size, D), np.float32)
    m = win < BATCH
    nbr[m] = np.asarray(src_nbr, np.float32)[win[m]]
    nbr[~m] = np.asarray(dst_nbr, np.float32)[win[~m] - BATCH]
    return uniq, nbr


def _split_bf16(x):
    import ml_dtypes
    hi = x.astype(ml_dtypes.bfloat16)
    lo = (x - hi.astype(np.float32)).astype(ml_dtypes.bfloat16)
    return hi, lo


def _wrap16(idx_zone):
    """[ZONE_CAP] int16 -> [128, IDX_COLS]: index i at (i%16, i//16),
    replicated down the 8 16-partition groups (one per Q7 core)."""
    blk = idx_zone.reshape(IDX_COLS, 16).T  # [16, IDX_COLS]
    return np.tile(blk, (8, 1))


def prepare_inputs(inputs):
    """Shard + route the full inputs into per-core in_maps.

    Returns (in_maps, spill, consts); spill lists (node_row, nbr_row)
    updates that exceeded a zone's capacity (normally empty), applied on
    the host afterwards."""
    prev_full = np.ascontiguousarray(
        np.asarray(inputs["previous_embedding"], np.float32))
    uniq, nbr = route_updates(
        inputs["src_node_ids"], inputs["dst_node_ids"],
        inputs["batch_src_neighbor_embedding"],
        inputs["batch_dst_neighbor_embedding"])

    w_nig = np.asarray(inputs["W_nig"], np.float64)
    b_nig = np.asarray(inputs["b_nig"], np.float64)
    w_node = np.asarray(inputs["W_node"], np.float64)
    b_node = np.asarray(inputs["b_node"], np.float64)
    wn = w_node.T.astype(np.float32)                  # [in, out]
    wc = (w_nig.T @ w_node.T).astype(np.float32)      # [in, out]
    bc = (b_nig @ w_node.T + b_node).astype(np.float32)
    wn_h, wn_l = _split_bf16(wn)
    wc_h, wc_l = _split_bf16(wc)
    bc_col = np.ascontiguousarray(bc.reshape(D, 1))

    in_maps = []
    spill = []
    # uniq is sorted -> contiguous runs per (core, zone)
    zone_of = uniq // RPZ  # global zone id 0..63
    bounds = np.searchsorted(zone_of, np.arange(N_CORES * N_ZONES + 1))
    for k in range(N_CORES):
        idx16 = np.empty((128, N_ZONES * IDX_COLS), np.int16)
        maskk = np.zeros(CAP, np.float32)
        nbrk = np.zeros((CAP, D), np.float32)
        for z in range(N_ZONES):
            zi = k * N_ZONES + z
            lo, hi = bounds[zi], bounds[zi + 1]
            n = hi - lo
            if n > ZONE_CAP:
                for r in range(lo + ZONE_CAP, hi):
                    spill.append((uniq[r], nbr[r]))
                n = ZONE_CAP
                hi = lo + n
            base = z * ZONE_CAP
            zidx = np.zeros(ZONE_CAP, np.int16)
            zidx[:n] = (uniq[lo:hi] - k * RPC - z * RPZ).astype(np.int16)
            idx16[:, z * IDX_COLS:(z + 1) * IDX_COLS] = _wrap16(zidx)
            maskk[base:base + n] = 1.0
            nbrk[base:base + n] = nbr[lo:hi]
        nb_h, nb_l = _split_bf16(np.ascontiguousarray(nbrk.T))
        in_maps.append({
            "prev": prev_full[k * RPC:(k + 1) * RPC],
            "nbh": nb_h, "nbl": nb_l,
            "idx": np.ascontiguousarray(idx16),
            "mask": np.ascontiguousarray(maskk.reshape(T_TILES, 128).T),
            "wnh": wn_h, "wnl": wn_l, "wch": wc_h, "wcl": wc_l,
            "bc": bc_col,
        })
    return in_maps, spill, (wn, wc, bc)


def assemble_output(results, spill, consts, prev_full):
    out = np.empty((N_NODES, D), np.float32)
    for k in range(N_CORES):
        for z in range(N_ZONES):
            out[k * RPC + z * RPZ:k * RPC + (z + 1) * RPZ] = \
                results[k][f"out{z}"]
    if spill:
        wn, wc, bc = consts
        for row, nbr_row in spill:
            out[row] = prev_full[row] + (prev_full[row] @ wn
                                         + nbr_row @ wc + bc)
    return out


def kernel(trace=False, **inputs):
    global last_results
    from concourse.bass_utils import run_bass_kernel_spmd

    nc = build_program()
    in_maps, spill, consts = prepare_inputs(inputs)
    res = run_bass_kernel_spmd(nc, in_maps, core_ids=list(range(N_CORES)),
                               trace=trace)
    last_results = res
    prev_full = np.asarray(inputs["previous_embedding"], np.float32)
    return assemble_output(res.results, spill, consts, prev_full)


# revision 10
# speedup vs baseline: 1.1466x; 1.1466x over previous
"""Trainium2 Bass kernel for BatchEmbeddingUpdater (gnn_message_passing).

Semantics replicated (matching the jax reference with in-order scatters):
    src_emb = (prev[src] + src_nbr @ W_nig.T + b_nig) @ W_node.T + b_node + prev[src]
    dst_emb = (prev[dst] + dst_nbr @ W_nig.T + b_nig) @ W_node.T + b_node + prev[dst]
    out = prev;  out[src] = src_emb;  out[dst] = dst_emb
(duplicates: LAST write wins within a batch; dst beats src — XLA/numpy
in-order scatter semantics)

Algebraic fusion (host precompute):
    out_row = prev_row + delta_row
    delta_row = prev_row @ Wn + nbr_row @ Wc + bc
    with Wn = W_node.T, Wc = W_nig.T @ W_node.T, bc = b_nig @ W_node.T + b_node

Sharding: previous_embedding row-partitioned across 8 cores (125k rows).
The ~181k winner updates are routed on host to the owning core; each core's
shard splits into 8 zones (15625 rows, separate output DRAM tensors) so
zone-local rows fit int16 for dma_scatter_add and the bulk copy ->
scatter-add ordering is per-zone.

Device data flow:
  - bulk DRAM->DRAM copy of each zone (out = prev)
  - per 512-update group: stream in pre-transposed bf16 hi/lo splits of the
    update rows (host-gathered) and neighbor rows; 6 wide bf16 matmuls into
    PSUM (Wn_h@Gh + Wn_h@Gl + Wn_l@Gh + Wc_h@Nh + Wc_h@Nl + Wc_l@Nh); +bc;
    PE transpose back to row-major; x mask (zeroes pad slots)
  - dma_scatter_add of the deltas onto the copied zone rows (exact f32
    "+prev" residual comes from the copied DRAM row)
bf16 hi+lo carries ~17 significand bits -> ~4e-6 relative error.
"""

import numpy as np

N_NODES = 1_000_000
BATCH = 100_000
D = 128
N_CORES = 8
RPC = N_NODES // N_CORES        # 125_000 rows per core
N_ZONES = 8
RPZ = RPC // N_ZONES            # 15_625 rows per zone (int16-addressable)
TILES_PER_ZONE = 24
ZONE_CAP = TILES_PER_ZONE * 128  # 3072 padded updates per zone
T_TILES = N_ZONES * TILES_PER_ZONE  # 192
CAP = N_ZONES * ZONE_CAP        # 24_576 updates per core (padded)
GRP = 4                          # tiles per matmul group (512 updates)
GROUPS_PER_ZONE = TILES_PER_ZONE // GRP
IDX_COLS = ZONE_CAP // 16        # 192 int16 idx columns per zone
COPY_CHUNKS = 2                 # bulk-copy DMAs per zone

_program = None
last_results = None  # BassKernelResults of the most recent kernel() call


def build_program():
    """Build + compile the (single, SPMD) Bass program. Cached."""
    global _program
    if _program is not None:
        return _program

    import concourse.mybir as mybir
    import concourse.tile as tile
    from concourse import bacc
    from concourse.masks import make_identity
    from concourse.tile_rust import add_dep_helper

    f32 = mybir.dt.float32
    bf16 = mybir.dt.bfloat16
    i16 = mybir.dt.int16
    ActFn = mybir.ActivationFunctionType

    nc = bacc.Bacc("TRN2", target_bir_lowering=False, debug=False,
                   num_devices=N_CORES)

    prev = nc.dram_tensor("prev", [RPC, D], f32, kind="ExternalInput").ap()
    gph_d = nc.dram_tensor("gph", [D, CAP], bf16, kind="ExternalInput").ap()
    gpl_d = nc.dram_tensor("gpl", [D, CAP], bf16, kind="ExternalInput").ap()
    nbh_d = nc.dram_tensor("nbh", [D, CAP], bf16, kind="ExternalInput").ap()
    nbl_d = nc.dram_tensor("nbl", [D, CAP], bf16, kind="ExternalInput").ap()
    idx_d = nc.dram_tensor("idx", [128, N_ZONES * IDX_COLS], i16,
                           kind="ExternalInput").ap()
    mask_d = nc.dram_tensor("mask", [128, T_TILES], f32,
                            kind="ExternalInput").ap()
    wn_d = [nc.dram_tensor(f"wn{s}", [D, D], bf16, kind="ExternalInput").ap()
            for s in "hl"]
    wc_d = [nc.dram_tensor(f"wc{s}", [D, D], bf16, kind="ExternalInput").ap()
            for s in "hl"]
    bc_d = nc.dram_tensor("bc", [D, 1], f32, kind="ExternalInput").ap()
    outs = [nc.dram_tensor(f"out{z}", [RPZ, D], f32, kind="ExternalOutput").ap()
            for z in range(N_ZONES)]

    with tile.TileContext(nc) as tc, \
         tc.tile_pool(name="const", bufs=1) as cpool, \
         tc.tile_pool(name="ins", bufs=3) as ipool, \
         tc.tile_pool(name="gt", bufs=2) as gtpool, \
         tc.tile_pool(name="outb", bufs=2) as opool, \
         tc.tile_pool(name="ps_b", bufs=4, space="PSUM") as psb, \
         tc.tile_pool(name="ps_a", bufs=3, space="PSUM") as psa:

        ident = cpool.tile([128, 128], f32, name="ident")
        make_identity(nc, ident[:])
        wn_sb = [cpool.tile([128, 128], bf16, name=f"wn{s}_sb") for s in "hl"]
        wc_sb = [cpool.tile([128, 128], bf16, name=f"wc{s}_sb") for s in "hl"]
        for d_ap, t in zip(wn_d + wc_d, wn_sb + wc_sb):
            nc.sync.dma_start(out=t[:], in_=d_ap)
        bc_sb = cpool.tile([128, 1], f32, name="bc_sb")
        nc.sync.dma_start(out=bc_sb[:], in_=bc_d)
        idx_sb = cpool.tile([128, N_ZONES * IDX_COLS], i16, name="idx_sb")
        nc.sync.dma_start(out=idx_sb[:], in_=idx_d)
        mask_sb = cpool.tile([128, T_TILES], f32, name="mask_sb")
        nc.sync.dma_start(out=mask_sb[:], in_=mask_d)

        # Bulk copy of the shard into the zone outputs (DRAM -> DRAM) on the
        # SP HWDGE ring. Tile does not track DRAM hazards; explicit
        # scatter-after-copy deps are added below.
        copy_insts = [[] for _ in range(N_ZONES)]
        rows_per_chunk = RPZ // COPY_CHUNKS
        for z in range(N_ZONES):
            for c in range(COPY_CHUNKS):
                r0 = c * rows_per_chunk
                r1 = RPZ if c == COPY_CHUNKS - 1 else r0 + rows_per_chunk
                ci = nc.sync.dma_start(out=outs[z][r0:r1, :],
                                       in_=prev[z * RPZ + r0:z * RPZ + r1, :])
                copy_insts[z].append(ci)

        for z in range(N_ZONES):
            ob = opool.tile([128, ZONE_CAP], f32, name="ob", tag="ob")
            for grp in range(GROUPS_PER_ZONE):
                t0 = z * TILES_PER_ZONE + grp * GRP  # global tile index
                us = slice(t0 * 128, t0 * 128 + GRP * 128)
                # stream the group's operands (ACT HWDGE ring)
                gph = ipool.tile([128, GRP * 128], bf16, name="gph", tag="gph")
                gpl = ipool.tile([128, GRP * 128], bf16, name="gpl", tag="gpl")
                nbh = ipool.tile([128, GRP * 128], bf16, name="nbh", tag="nbh")
                nbl = ipool.tile([128, GRP * 128], bf16, name="nbl", tag="nbl")
                nc.scalar.dma_start(out=gph[:], in_=gph_d[:, us])
                nc.scalar.dma_start(out=gpl[:], in_=gpl_d[:, us])
                nc.scalar.dma_start(out=nbh[:], in_=nbh_d[:, us])
                nc.scalar.dma_start(out=nbl[:], in_=nbl_d[:, us])
                acc = psa.tile([128, GRP * 128], f32, name="acc", tag="acc")
                nc.tensor.matmul(acc[:], lhsT=wn_sb[0][:], rhs=gph[:],
                                 start=True, stop=False)
                nc.tensor.matmul(acc[:], lhsT=wn_sb[0][:], rhs=gpl[:],
                                 start=False, stop=False)
                nc.tensor.matmul(acc[:], lhsT=wn_sb[1][:], rhs=gph[:],
                                 start=False, stop=False)
                nc.tensor.matmul(acc[:], lhsT=wc_sb[0][:], rhs=nbh[:],
                                 start=False, stop=False)
                nc.tensor.matmul(acc[:], lhsT=wc_sb[0][:], rhs=nbl[:],
                                 start=False, stop=False)
                nc.tensor.matmul(acc[:], lhsT=wc_sb[1][:], rhs=nbh[:],
                                 start=False, stop=True)
                outt = gtpool.tile([128, GRP * 128], f32, name="outt",
                                   tag="outt")
                nc.vector.tensor_scalar_add(outt[:], acc[:], bc_sb[:, :1])
                for j in range(GRP):
                    c0 = (grp * GRP + j) * 128
                    tb = psb.tile([128, 128], f32, name="tb", tag="tb")
                    nc.tensor.transpose(tb[:], outt[:, j * 128:(j + 1) * 128],
                                        ident[:])
                    # masked move (mask: 1.0 real updates, 0.0 pads),
                    # alternating DVE / ACT to split the PSUM-read load
                    mcol = mask_sb[:, t0 + j:t0 + j + 1]
                    if j % 2 == 0:
                        nc.vector.tensor_scalar_mul(ob[:, c0:c0 + 128], tb[:],
                                                    mcol)
                    else:
                        nc.scalar.activation(ob[:, c0:c0 + 128], tb[:],
                                             ActFn.Copy, scale=mcol)
            # Scatter-add the zone's deltas onto the copied rows.
            sc = nc.gpsimd.dma_scatter_add(
                out_ap=outs[z],
                in_ap=ob[:].rearrange("p (c e) -> p c e", e=128),
                idxs_ap=idx_sb[:, z * IDX_COLS:(z + 1) * IDX_COLS],
                num_idxs=ZONE_CAP, num_idxs_reg=ZONE_CAP, elem_size=128,
                single_packet=False,
            )
            for ci in copy_insts[z]:
                add_dep_helper(sc.ins, ci.ins, sync=True,
                               reason="scatter-add after zone bulk copy")

    nc.compile()
    _program = nc
    return nc


def route_updates(src_ids, dst_ids, src_nbr, dst_nbr):
    """Dedup the two scatter batches into winner updates (last wins, dst
    over src) and return (uniq_node_ids_sorted, winner_nbr_rows)."""
    ids = np.concatenate([np.asarray(src_ids, np.int64),
                          np.asarray(dst_ids, np.int64)])
    rev = ids[::-1]
    uniq, idx_rev = np.unique(rev, return_index=True)
    win = ids.size - 1 - idx_rev        # winning write position
    nbr = np.empty((uniq.size, D), np.float32)
    m = win < BATCH
    nbr[m] = np.asarray(src_nbr, np.float32)[win[m]]
    nbr[~m] = np.asarray(dst_nbr, np.float32)[win[~m] - BATCH]
    return uniq, nbr


def _split_bf16(x):
    import ml_dtypes
    hi = x.astype(ml_dtypes.bfloat16)
    lo = (x - hi.astype(np.float32)).astype(ml_dtypes.bfloat16)
    return hi, lo


def _wrap16(idx_zone):
    """[ZONE_CAP] int16 -> [128, IDX_COLS]: index i at (i%16, i//16),
    replicated down the 8 16-partition groups (one per Q7 core)."""
    blk = idx_zone.reshape(IDX_COLS, 16).T  # [16, IDX_COLS]
    return np.tile(blk, (8, 1))


def prepare_inputs(inputs):
    """Shard + route the full inputs into per-core in_maps.

    Returns (in_maps, spill, consts); spill lists (node_row, nbr_row)
    updates that exceeded a zone's capacity (normally empty), applied on
    the host afterwards."""
    prev_full = np.ascontiguousarray(
        np.asarray(inputs["previous_embedding"], np.float32))
    uniq, nbr = route_updates(
        inputs["src_node_ids"], inputs["dst_node_ids"],
        inputs["batch_src_neighbor_embedding"],
        inputs["batch_dst_neighbor_embedding"])

    w_nig = np.asarray(inputs["W_nig"], np.float64)
    b_nig = np.asarray(inputs["b_nig"], np.float64)
    w_node = np.asarray(inputs["W_node"], np.float64)
    b_node = np.asarray(inputs["b_node"], np.float64)
    wn = w_node.T.astype(np.float32)                  # [in, out]
    wc = (w_nig.T @ w_node.T).astype(np.float32)      # [in, out]
    bc = (b_nig @ w_node.T + b_node).astype(np.float32)
    wn_h, wn_l = _split_bf16(wn)
    wc_h, wc_l = _split_bf16(wc)
    bc_col = np.ascontiguousarray(bc.reshape(D, 1))

    in_maps = []
    spill = []
    # uniq is sorted -> contiguous runs per (core, zone)
    zone_of = uniq // RPZ  # global zone id 0..63
    bounds = np.searchsorted(zone_of, np.arange(N_CORES * N_ZONES + 1))
    for k in range(N_CORES):
        idx16 = np.empty((128, N_ZONES * IDX_COLS), np.int16)
        maskk = np.zeros(CAP, np.float32)
        nbrk = np.zeros((CAP, D), np.float32)
        gpk = np.zeros((CAP, D), np.float32)
        for z in range(N_ZONES):
            zi = k * N_ZONES + z
            lo, hi = bounds[zi], bounds[zi + 1]
            n = hi - lo
            if n > ZONE_CAP:
                for r in range(lo + ZONE_CAP, hi):
                    spill.append((uniq[r], nbr[r]))
                n = ZONE_CAP
                hi = lo + n
            base = z * ZONE_CAP
            zidx = np.zeros(ZONE_CAP, np.int16)
            zidx[:n] = (uniq[lo:hi] - k * RPC - z * RPZ).astype(np.int16)
            idx16[:, z * IDX_COLS:(z + 1) * IDX_COLS] = _wrap16(zidx)
            maskk[base:base + n] = 1.0
            nbrk[base:base + n] = nbr[lo:hi]
            gpk[base:base + n] = prev_full[uniq[lo:hi]]
        nb_h, nb_l = _split_bf16(np.ascontiguousarray(nbrk.T))
        gp_h, gp_l = _split_bf16(np.ascontiguousarray(gpk.T))
        in_maps.append({
            "prev": prev_full[k * RPC:(k + 1) * RPC],
            "gph": gp_h, "gpl": gp_l,
            "nbh": nb_h, "nbl": nb_l,
            "idx": np.ascontiguousarray(idx16),
            "mask": np.ascontiguousarray(maskk.reshape(T_TILES, 128).T),
            "wnh": wn_h, "wnl": wn_l, "wch": wc_h, "wcl": wc_l,
            "bc": bc_col,
        })
    return in_maps, spill, (wn, wc, bc)


def assemble_output(results, spill, consts, prev_full):
    out = np.empty((N_NODES, D), np.float32)
    for k in range(N_CORES):
        for z in range(N_ZONES):
            out[k * RPC + z * RPZ:k * RPC + (z + 1) * RPZ] = \
                results[k][f"out{z}"]
    if spill:
        wn, wc, bc = consts
        for row, nbr_row in spill:
            out[row] = prev_full[row] + (prev_full[row] @ wn
                                         + nbr_row @ wc + bc)
    return out


def kernel(trace=False, **inputs):
    global last_results
    from concourse.bass_utils import run_bass_kernel_spmd

    nc = build_program()
    in_maps, spill, consts = prepare_inputs(inputs)
    res = run_bass_kernel_spmd(nc, in_maps, core_ids=list(range(N_CORES)),
                               trace=trace)
    last_results = res
    prev_full = np.asarray(inputs["previous_embedding"], np.float32)
    return assemble_output(res.results, spill, consts, prev_full)
